# revision 9
# baseline (speedup 1.0000x reference)
"""Chamfer distance loss kernel for Trainium2 (8 NeuronCores, SPMD).

Problem: pred (4, 8192, 3) f32, target (4, 8192, 3) f32.
loss = mean_n min_m ||p_n - t_m||^2 + mean_m min_n ||p_n - t_m||^2

Sharding: 8 cores = 4 batches x 2 pred-row halves. Each core computes the
(4096 x 8192) block of the squared-distance matrix for its (batch, half):
  - row minima over all 8192 targets (exact dist1 contributions)
  - running column minima over its 4096 pred rows (partial dist2)
Host combines: sums row minima; mins the two column-minima vectors per batch.

Device pipeline per core:
  - PE: d = p2 + t2 - 2 p.t as K=18 (padded to 32) matmuls, 4 row chunks
    packed into the four 32-row PE groups via tile_position (concurrent MMs).
    bf16 hi/lo split of p, t and 3-way bf16 split of the squared norms keeps
    fp32-level accuracy; PSUM accumulates fp32.
  - ACT: evacuate PSUM -> SBUF with bf16 cast (1024-wide).
  - DVE: colmin accumulate (tensor_tensor min, 2x bf16) + rowmin via
    tensor_tensor_reduce (fold halves, fused free-axis min into accum_out).
  - PE transpose + DVE reduce collapse the colmin partition axis.
"""

import os
import sys

for _p in ("/opt/trn_rl_repo", "/opt/pypackages"):
    if _p not in sys.path:
        sys.path.insert(0, _p)

import numpy as np
import ml_dtypes

BF16 = ml_dtypes.bfloat16

B = 4
N = 8192  # pred points per batch
M = 8192  # target points per batch
HALF = N // 2  # pred rows per core = 4096
NCORES = 8
K = 18  # live contraction rows of the augmented matmul
PK = 32  # padded rows per PE row-group
RCHUNKS = HALF // 128  # 32 row chunks of 128 partitions
NG = 4  # row chunks processed concurrently via PE row groups
JG = RCHUNKS // NG  # 8 chunk groups
FT = 512  # matmul free-dim tile (one PSUM bank of fp32)
PW = 1024  # PSUM tile width per chunk (2 banks); ACT evacuation width

USE_TTR = os.environ.get("CHAMFER_TTR", "1") == "1"

_compiled = None


def _bf(x):
    return x.astype(BF16)


def _split3(x64):
    """3-way bf16 split of a float64 array; sum of parts ~ x to ~2^-24."""
    a = _bf(x64)
    r = x64 - a.astype(np.float64)
    b = _bf(r)
    r = r - b.astype(np.float64)
    c = _bf(r)
    return a, b, c


def _prep_lhs(p):
    """pred half (HALF, 3) f32 -> predAug (K, HALF) bf16."""
    p64 = p.astype(np.float64)
    ph = _bf(p)
    pl = _bf((p64 - ph.astype(np.float64)).astype(np.float32))
    p2 = (p64 * p64).sum(-1)
    a, b, c = _split3(p2)
    out = np.empty((K, p.shape[0]), dtype=BF16)
    out[0:3] = ph.T
    out[3:6] = pl.T
    out[6:9] = ph.T
    out[9:12] = pl.T
    out[12] = a
    out[13] = b
    out[14] = c
    out[15:18] = BF16(1.0)
    return out


def _prep_rhs(t):
    """target (M, 3) f32 -> targAug (K, M) bf16."""
    t64 = t.astype(np.float64)
    th = _bf(t)
    tl = _bf((t64 - th.astype(np.float64)).astype(np.float32))
    t2 = (t64 * t64).sum(-1)
    a, b, c = _split3(t2)
    out = np.empty((K, t.shape[0]), dtype=BF16)
    out[0:3] = (-2.0 * th.astype(np.float32)).astype(BF16).T
    out[3:6] = out[0:3]
    out[6:9] = (-2.0 * tl.astype(np.float32)).astype(BF16).T
    out[9:12] = out[6:9]
    out[12:15] = BF16(1.0)
    out[15] = a
    out[16] = b
    out[17] = c
    return out


def _build_program():
    import concourse.tile as tile
    from concourse import bacc, mybir

    nc = bacc.Bacc("TRN2", target_bir_lowering=False, debug=False, num_devices=NCORES)
    dt = mybir.dt
    Alu = mybir.AluOpType
    Ax = mybir.AxisListType

    pa_d = nc.dram_tensor(
        "pred_aug4", [128, JG * 128], dt.bfloat16, kind="ExternalInput"
    ).ap()
    ta_d = nc.dram_tensor("targ_aug4", [128, M], dt.bfloat16, kind="ExternalInput").ap()
    id_d = nc.dram_tensor("ident", [128, 128], dt.bfloat16, kind="ExternalInput").ap()
    rm_d = nc.dram_tensor(
        "rowmins", [128, RCHUNKS], dt.float32, kind="ExternalOutput"
    ).ap()
    ct_d = nc.dram_tensor(
        "colminT", [128, M // 128], dt.float32, kind="ExternalOutput"
    ).ap()

    with tile.TileContext(nc) as tc:
        with (
            tc.tile_pool(name="consts", bufs=1) as consts,
            tc.tile_pool(name="dchunk", bufs=6) as dpool,
            tc.tile_pool(name="psum", bufs=4, space="PSUM") as psum,
        ):
            pa = consts.tile([128, JG * 128], dt.bfloat16)
            ta = consts.tile([128, M], dt.bfloat16)
            ident = consts.tile([128, 128], dt.bfloat16)
            cm = consts.tile([128, M], dt.bfloat16)  # colmin accumulator
            rm = consts.tile([128, RCHUNKS], dt.float32)
            ct = consts.tile([128, M // 128], dt.float32)
            dummy = consts.tile([128, 1], dt.bfloat16)

            nc.sync.dma_start(pa[:], pa_d[:])
            nc.sync.dma_start(ta[:], ta_d[:])
            nc.sync.dma_start(ident[:], id_d[:])
            nc.vector.memset(cm[:], 1e30)

            for j in range(JG):
                dcs = [dpool.tile([128, M], dt.bfloat16, tag="dc", name=f"dc_{j}_{i}") for i in range(NG)]
                for h in range(M // PW):
                    pts = [
                        psum.tile([128, PW], dt.float32, tag="mm", name=f"pt_{j}_{h}_{i}")
                        for i in range(NG)
                    ]
                    for s in range(PW // FT):
                        for i in range(NG):
                            f0 = h * PW + s * FT
                            nc.tensor.matmul(
                                pts[i][:, s * FT:(s + 1) * FT],
                                lhsT=pa[32 * i:32 * i + PK, j * 128:(j + 1) * 128],
                                rhs=ta[32 * i:32 * i + PK, f0:f0 + FT],
                                start=True,
                                stop=True,
                                tile_position=(32 * i, 0),
                            )
                    for i in range(NG):
                        nc.scalar.copy(dcs[i][:, h * PW:(h + 1) * PW], pts[i][:])
                for i in range(NG):
                    r = j * NG + i
                    dc = dcs[i]
                    # column-minima accumulate (dist2 side), 2x bf16 mode
                    nc.vector.tensor_tensor(cm[:], dc[:], cm[:], op=Alu.min)
                    if USE_TTR:
                        # rowmin: fold halves + fused free-axis min-reduce
                        nc.vector.tensor_tensor_reduce(
                            dummy[:].broadcast_to((128, M // 2)),
                            dc[:, :M // 2],
                            dc[:, M // 2:],
                            scale=1.0,
                            scalar=1e30,
                            op0=Alu.min,
                            op1=Alu.min,
                            accum_out=rm[:, r:r + 1],
                        )
                    else:
                        w = M // 2
                        while w >= 64:
                            nc.vector.tensor_tensor(
                                dc[:, :w], dc[:, :w], dc[:, w:2 * w], op=Alu.min
                            )
                            w //= 2
                        nc.vector.tensor_reduce(
                            rm[:, r:r + 1], dc[:, :128], axis=Ax.X, op=Alu.min
                        )

            # collapse colmin partition axis: PE transpose 128x128 blocks,
            # then min-reduce the (former partition) free axis.
            nblk = M // 128  # 64
            for g in range(nblk // 4):
                ptt = psum.tile([128, 4, 128], dt.bfloat16, tag="mm")
                for t in range(4):
                    blk = g * 4 + t
                    nc.tensor.transpose(
                        ptt[:, t, :], cm[:, blk * 128:(blk + 1) * 128], ident[:]
                    )
                nc.vector.tensor_reduce(
                    ct[:, g * 4:(g + 1) * 4], ptt[:], axis=Ax.X, op=Alu.min
                )

            nc.sync.dma_start(rm_d[:], rm[:])
            nc.sync.dma_start(ct_d[:], ct[:])

    nc.compile()
    return nc


def _get_program():
    global _compiled
    if _compiled is None:
        _compiled = _build_program()
    return _compiled


def make_in_maps(pred, target):
    """Build the per-core input dicts from full inputs."""
    pred = np.asarray(pred, dtype=np.float32)
    target = np.asarray(target, dtype=np.float32)
    ident = np.eye(128, dtype=BF16)
    in_maps = []
    for c in range(NCORES):
        b, half = divmod(c, 2)
        p = pred[b, half * HALF:(half + 1) * HALF]
        la = _prep_lhs(p)  # (K, HALF)
        ra = _prep_rhs(target[b])  # (K, M)

        # pack row chunks into the 4 PE row groups:
        # pa4[32*i + k, j*128 + c] = la[k, (NG*j + i)*128 + c], rows K..31 zero
        pa4 = np.zeros((128, JG * 128), dtype=BF16)
        lc = la.reshape(K, RCHUNKS, 128)  # (K, r, c)
        for i in range(NG):
            # chunks r = NG*j + i for j in 0..JG-1
            pa4[32 * i:32 * i + K] = lc[:, i::NG, :].reshape(K, JG * 128)
        # replicate targets into all 4 row groups, rows K..31 zero
        ta4 = np.zeros((128, M), dtype=BF16)
        for i in range(NG):
            ta4[32 * i:32 * i + K] = ra
        in_maps.append({"pred_aug4": pa4, "targ_aug4": ta4, "ident": ident})
    return in_maps


def combine(results):
    """Combine per-core outputs into the scalar loss."""
    d1 = 0.0
    d2 = 0.0
    for b in range(B):
        r0, r1 = results[2 * b], results[2 * b + 1]
        d1 += r0["rowmins"].astype(np.float64).sum()
        d1 += r1["rowmins"].astype(np.float64).sum()
        cmin = np.minimum(r0["colminT"], r1["colminT"]).astype(np.float64)
        d2 += cmin.sum()
    loss = d1 / (B * N) + d2 / (B * M)
    return np.float32(loss)


def kernel(pred, target):
    from concourse.bass_utils import run_bass_kernel_spmd

    nc = _get_program()
    in_maps = make_in_maps(pred, target)
    res = run_bass_kernel_spmd(nc, in_maps, list(range(NCORES)))
    return np.asarray(combine(res.results))
